# revision 41
# baseline (speedup 1.0000x reference)
"""Causal multi-head attention (B=4,T=2048,C=1024,H=16,D=64) on 8 TRN2 cores.

Sharding: core = 2*b + hg  (b = batch 0..3, hg = head-group 0..1, 8 heads each).
Each core computes, for its batch b and its 8 heads:
  QT,KT  = (x_b @ Wq|Wk)^T      via matmul(lhsT=w_cols, rhs=x_b^T)
  V      = x_b @ Wv             (natural layout, lhsT=x_b^T tiles)
  pT     = exp(KT_h^T Q_h / 8)  (transposed scores, causal blocks only,
                                 no max-subtraction: scores are ~N(0,1/3))
  [yT;l] = [V|1]^T @ pT         (fused attention output + softmax denom)
  yT_n   = yT * (1/l)           (broadcast + DVE multiply)
  part   = Y^T.T @ Wproj_rows   (yT is directly the lhsT for the projection)
Host: out[b] = part[2b] + part[2b+1] + b_proj  (tensor-parallel unshard).

Head pairs (2j, 2j+1) are processed together: their K=64 score matmuls sit
at partition bases 0/64 (disjoint PE row groups, disjoint PSUM banks) so
they run concurrently, and one wide ACT does exp for both. The AV matmuls
are deferred one block behind score+exp (carried across pipeline
boundaries) so neither engine drains. QKV/V matmul units and projection
tile-groups are interleaved into the attention stream just-in-time, which
keeps the TensorEngine -- the binding resource at ~216ns per 128x512 bf16
matmul -- issue-saturated from the first DMA arrival to the tail.
"""

import sys

sys.path.insert(0, "/opt/trn_rl_repo")

import numpy as np
import ml_dtypes

B, T, C = 4, 2048, 1024
H, D = 16, 64
NCORES = 8
HPC = 8  # heads per core

TRACE = False
LAST_EXEC_NS = None

_cache = {}


def _build():
    if "nc" in _cache:
        return _cache["nc"]
    import concourse.bass as bass  # noqa: F401
    import concourse.mybir as mybir
    from concourse import bacc, tile

    bf16 = mybir.dt.bfloat16
    f32 = mybir.dt.float32
    AF = mybir.ActivationFunctionType

    nc = bacc.Bacc(
        "TRN2", target_bir_lowering=False, debug=False, num_devices=NCORES
    )

    xT = nc.declare_dram_parameter("xT", [C, T], bf16, isOutput=False)
    wqk = nc.declare_dram_parameter("wqk", [C, 1024], bf16, isOutput=False)
    wv = nc.declare_dram_parameter("wv", [C, 512], bf16, isOutput=False)
    wp = nc.declare_dram_parameter("wp", [512, C], bf16, isOutput=False)
    mk = nc.declare_dram_parameter("mk", [128, 128], bf16, isOutput=False)
    out = nc.declare_dram_parameter("out", [T, C], f32, isOutput=True)

    KT = C // 128  # 8 contraction tiles for qkv
    TT = T // 128  # 16 s-blocks / t-tiles
    NR = T // 512  # 4 t-ranges of 512

    with tile.TileContext(nc) as tc:
        with (
            tc.tile_pool(name="wpool", bufs=1) as wpool,
            tc.tile_pool(name="big", bufs=1) as big,
            tc.tile_pool(name="pwork", bufs=4) as pwork,
            tc.tile_pool(name="owork", bufs=3) as owork,
            tc.tile_pool(name="rwork", bufs=3) as rwork,
            tc.tile_pool(name="psA", bufs=3, space="PSUM") as psA,
            tc.tile_pool(name="psY", bufs=1, space="PSUM") as psY,
        ):
            dma = nc.default_dma_engine

            # ---- loads (k-interleaved so the first matmuls start early) ----
            mask = wpool.tile([128, 128], bf16, tag="mask")
            dma.dma_start(mask[:], mk[:])
            # preload the exp table-set during startup DMA
            dum = rwork.tile([1, 8], bf16, tag="dum")
            nc.scalar.activation(dum[0:1, :], mask[0:1, 0:8], AF.Exp)
            xt, wqk_t, wv_t = [], [], []
            xTr = xT.rearrange("(k p) t -> k p t", p=128)
            wqkr = wqk.rearrange("(k p) m -> k p m", p=128)
            wvr = wv.rearrange("(k p) m -> k p m", p=128)
            for k in range(KT):
                # spread the startup loads across three queue engines --
                # a single HWDGE queue (~20 GB/s) would pace the first
                # third of the kernel
                t_ = wpool.tile([128, T], bf16, tag=f"xt{k}", name=f"xt{k}")
                dma.dma_start(t_[:], xTr[k])
                xt.append(t_)
                t_ = wpool.tile([128, 1024], bf16, tag=f"wqk{k}", name=f"wqk{k}")
                dma.dma_start(t_[:], wqkr[k])
                wqk_t.append(t_)
                t_ = wpool.tile([128, 512], bf16, tag=f"wv{k}", name=f"wv{k}")
                dma.dma_start(t_[:], wvr[k])
                wv_t.append(t_)
            wp_t = []
            wpr = wp.rearrange("(k p) m -> k p m", p=128)
            for k in range(4):
                t_ = wpool.tile([128, 1024], bf16, tag=f"wp{k}", name=f"wp{k}")
                dma.dma_start(t_[:], wpr[k])
                wp_t.append(t_)

            # ---- QT/KT per-m tiles (m 0-3 = Q heads 2m,2m+1; 4-7 = K) and
            # V-with-ones per-tm2 tiles are emitted as units, interleaved
            # into the attention stream below so the Scalar engine (exp)
            # starts early and QKV matmuls fill the PE slack.
            qkt_t = [
                big.tile([128, T], bf16, tag=f"qkt{m}", name=f"qkt{m}")
                for m in range(8)
            ]
            vones_t = [
                big.tile(
                    [128, 2, HPC, 65], bf16, tag=f"vones{tm2}", name=f"vones{tm2}"
                )
                for tm2 in range(TT // 2)
            ]

            def emit_qkt_half(m, n2):
                # one psum tile; 2 consecutive matmuls share each stationary
                # weight (k-outer) so LDWEIGHTS bubbles amortize
                qm = qkt_t[m]
                ps = psA.tile([128, 1024], f32, tag="ps", name=f"qk{m}{n2}")
                for k in range(KT):
                    for half in range(2):
                        n = 2 * n2 + half
                        nc.tensor.matmul(
                            ps[:, 512 * half : 512 * half + 512],
                            wqk_t[k][:, m * 128 : (m + 1) * 128],
                            xt[k][:, n * 512 : (n + 1) * 512],
                            start=(k == 0),
                            stop=(k == KT - 1),
                        )
                nc.vector.tensor_copy(
                    qm[:, n2 * 1024 : (n2 + 1) * 1024], ps[:]
                )

            def emit_v(tm2):
                vt = vones_t[tm2]
                nc.gpsimd.memset(vt[:, :, :, 64], 1.0)
                ps = psA.tile([128, 1024], f32, tag="ps", name=f"v{tm2}")
                for k in range(KT):
                    for half in range(2):
                        tm = 2 * tm2 + half
                        nc.tensor.matmul(
                            ps[:, 512 * half : 512 * half + 512],
                            xt[k][:, tm * 128 : (tm + 1) * 128],
                            wv_t[k][:],
                            start=(k == 0),
                            stop=(k == KT - 1),
                        )
                for half in range(2):
                    nc.vector.tensor_copy(
                        vt[:, half, :, 0:64],
                        ps[:, 512 * half : 512 * half + 512].rearrange(
                            "p (h d) -> p h d", h=HPC
                        ),
                    )

            # ---- attention + interleaved projection ----
            # yt_n[n][64*(h%2)+d, h//2, tl] = y_h[512n+tl, d] / l_h[512n+tl]
            yt_n = [
                big.tile([128, 4, 512], bf16, tag=f"ytn{n}", name=f"ytn{n}")
                for n in range(NR)
            ]

            proj_queue = []

            def mk_proj(n, tl):
                def f():
                    tt = 4 * n + tl
                    pp = psA.tile([128, 1024], f32, tag="ps", name=f"prj{tt}")
                    for n2 in range(2):
                        for k4 in range(4):
                            nc.tensor.matmul(
                                pp[:, 512 * n2 : 512 * n2 + 512],
                                yt_n[n][:, k4, tl * 128 : (tl + 1) * 128],
                                wp_t[k4][:, n2 * 512 : (n2 + 1) * 512],
                                start=(k4 == 0),
                                stop=(k4 == 3),
                            )
                    o_s = owork.tile([128, 1024], f32, tag="osb", name=f"os{tt}")
                    nc.vector.tensor_copy(o_s[:], pp[:])
                    dma.dma_start(out[tt * 128 : (tt + 1) * 128, :], o_s[:])

                return f

            blk = {"count": 0}

            def maybe_proj():
                if proj_queue and blk["count"] % 8 == 0:
                    proj_queue.pop(0)()

            defer = []  # deferred AV-matmul / normalization emissions that
            # carry the score->AV skew across pipeline boundaries so the PE
            # never drains waiting for the last exp of a pipeline

            def att(n, j):
                if True:
                    mq = j
                    mk_ = 4 + j
                    ys = [
                        psY.tile([65, 512], f32, tag=f"yt{hh}", name=f"ys{hh}")
                        for hh in range(2)
                    ]
                    nsb = 4 * n + 4
                    for sb in range(nsb):
                        smin = 128 * sb
                        t0 = max(0, smin - 512 * n)
                        ps = psA.tile([128, 1024], f32, tag="ps")
                        for hh in range(2):
                            pq = 64 * hh
                            nc.tensor.matmul(
                                ps[:, 512 * hh + t0 : 512 * hh + 512],
                                qkt_t[mk_][pq : pq + 64, smin : smin + 128],
                                qkt_t[mq][
                                    pq : pq + 64,
                                    512 * n + t0 : 512 * (n + 1),
                                ],
                                start=True,
                                stop=True,
                                skip_group_check=True,
                            )
                        p = pwork.tile([128, 1024], bf16, tag="p")
                        nc.scalar.activation(
                            p[:].rearrange("q (hh t) -> q hh t", hh=2)[
                                :, :, t0:512
                            ],
                            ps[:].rearrange("q (hh t) -> q hh t", hh=2)[
                                :, :, t0:512
                            ],
                            AF.Exp,
                            scale=0.125,
                        )
                        if sb // 4 == n:
                            # diagonal block: zero s>t half (exact, post-exp)
                            nc.vector.tensor_mul(
                                p[:].rearrange("q (hh t) -> q hh t", hh=2)[
                                    :, :, t0 : t0 + 128
                                ],
                                p[:].rearrange("q (hh t) -> q hh t", hh=2)[
                                    :, :, t0 : t0 + 128
                                ],
                                mask[:, None, :].broadcast_to([128, 2, 128]),
                            )
                        while defer:
                            defer.pop(0)()

                        def mk_yt(sb_, t0_, p_, ys_, j_, n_):
                            def f():
                                for hh in range(2):
                                    nc.tensor.matmul(
                                        ys_[hh][:, t0_:512],
                                        vones_t[sb_ // 2][
                                            :, sb_ % 2, 2 * j_ + hh, :
                                        ],
                                        p_[:, 512 * hh + t0_ : 512 * hh + 512],
                                        start=(sb_ == 0),
                                        stop=(sb_ == 4 * n_ + 3),
                                        skip_group_check=True,
                                    )
                                blk["count"] += 1
                                maybe_proj()

                            return f

                        defer.append(mk_yt(sb, t0, p, ys, j, n))
                    defer.append(mk_norm(n, j, ys))

            def mk_norm(n, j, ys):
                def f():
                    for hh in range(2):
                        # free the PSUM bank fast with a copy, then broadcast
                        # l, reciprocal on 64 lanes, normalize -- all off the
                        # PE/ACT critical path
                        ysb = rwork.tile([65, 512], f32, tag="ysb")
                        nc.vector.tensor_copy(ysb[:], ys[hh][:])
                        ls = rwork.tile([1, 512], f32, tag="ls")
                        nc.vector.tensor_copy(ls[0:1, :], ysb[64:65, :])
                        lr = rwork.tile([1, 512], f32, tag="lr")
                        nc.vector.reciprocal_approx_fast(
                            out=lr[0:1, :], in_=ls[0:1, :]
                        )
                        rb = rwork.tile([64, 512], f32, tag="rb")
                        nc.gpsimd.partition_broadcast(rb[:], lr[0:1, :])
                        nc.vector.tensor_mul(
                            yt_n[n][64 * hh : 64 * hh + 64, j, :],
                            ysb[0:64, :],
                            rb[:],
                        )

                return f

            # Emission order: feed each attention pipeline just in time.
            # att(0/1,*) touch only the first 1024 columns of QT/KT, so the
            # n2=1 QKT halves and late V units are deferred into the late,
            # ACT-paced region where the PE has slack.
            for j in range(4):
                emit_qkt_half(j, 0)
                emit_qkt_half(4 + j, 0)
                if j < 2:
                    emit_v(2 * j)
                    emit_v(2 * j + 1)
                att(0, j)
                if j >= 1:
                    att(1, j - 1)
            att(1, 3)
            while defer:
                defer.pop(0)()
            for tl in range(4):
                proj_queue.append(mk_proj(0, tl))
                proj_queue.append(mk_proj(1, tl))
            for j in range(4):
                if j == 0:
                    emit_v(4)
                    emit_v(5)
                emit_qkt_half(j, 1)
                emit_qkt_half(4 + j, 1)
                att(2, j)
            while defer:
                defer.pop(0)()
            for tl in range(4):
                proj_queue.append(mk_proj(2, tl))
            for j in range(4):
                if j == 0:
                    emit_v(6)
                    emit_v(7)
                att(3, j)
            while defer:
                defer.pop(0)()
            for tl in range(4):
                proj_queue.append(mk_proj(3, tl))
            while proj_queue:
                proj_queue.pop(0)()

    nc.compile()
    _cache["nc"] = nc
    return nc


def kernel(x, w_qkv, w_proj, b_proj):
    global LAST_EXEC_NS
    from concourse.bass_utils import run_bass_kernel_spmd

    x = np.asarray(x)
    w_qkv = np.asarray(w_qkv)
    w_proj = np.asarray(w_proj)
    b_proj = np.asarray(b_proj)

    nc = _build()
    bf = ml_dtypes.bfloat16
    # mask[s, t] = 1 where t >= s (keep), 0 where s > t (causal-masked)
    maskt = np.triu(np.ones((128, 128), np.float32)).astype(bf)

    in_maps = []
    for core in range(NCORES):
        b, hg = core // 2, core % 2
        cs = 512 * hg
        in_maps.append(
            {
                "xT": np.ascontiguousarray(x[b].T.astype(bf)),
                "wqk": np.ascontiguousarray(
                    np.concatenate(
                        [w_qkv[:, cs : cs + 512], w_qkv[:, 1024 + cs : 1536 + cs]],
                        axis=1,
                    ).astype(bf)
                ),
                "wv": np.ascontiguousarray(
                    w_qkv[:, 2048 + cs : 2560 + cs].astype(bf)
                ),
                "wp": np.ascontiguousarray(w_proj[cs : cs + 512, :].astype(bf)),
                "mk": maskt,
            }
        )

    res = run_bass_kernel_spmd(nc, in_maps, list(range(NCORES)), trace=TRACE)
    LAST_EXEC_NS = res.exec_time_ns
    results = res.results

    outv = np.empty((B, T, C), np.float32)
    for b in range(B):
        outv[b] = (
            results[2 * b]["out"].astype(np.float32)
            + results[2 * b + 1]["out"].astype(np.float32)
            + b_proj[None, :].astype(np.float32)
        )
    return outv


# revision 42
# speedup vs baseline: 1.0007x; 1.0007x over previous
"""Causal multi-head attention (B=4,T=2048,C=1024,H=16,D=64) on 8 TRN2 cores.

Sharding: core = 2*b + hg  (b = batch 0..3, hg = head-group 0..1, 8 heads each).
Each core computes, for its batch b and its 8 heads:
  QT,KT  = (x_b @ Wq|Wk)^T      via matmul(lhsT=w_cols, rhs=x_b^T)
  V      = x_b @ Wv             (natural layout, lhsT=x_b^T tiles)
  pT     = exp(KT_h^T Q_h / 8)  (transposed scores, causal blocks only,
                                 no max-subtraction: scores are ~N(0,1/3))
  [yT;l] = [V|1]^T @ pT         (fused attention output + softmax denom)
  yT_n   = yT * (1/l)           (broadcast + DVE multiply)
  part   = Y^T.T @ Wproj_rows   (yT is directly the lhsT for the projection)
Host: out[b] = part[2b] + part[2b+1] + b_proj  (tensor-parallel unshard).

Head pairs (2j, 2j+1) are processed together: their K=64 score matmuls sit
at partition bases 0/64 (disjoint PE row groups, disjoint PSUM banks) so
they run concurrently, and one wide ACT does exp for both. The AV matmuls
are deferred one block behind score+exp (carried across pipeline
boundaries) so neither engine drains. QKV/V matmul units and projection
tile-groups are interleaved into the attention stream just-in-time, which
keeps the TensorEngine -- the binding resource at ~216ns per 128x512 bf16
matmul -- issue-saturated from the first DMA arrival to the tail.
"""

import sys

sys.path.insert(0, "/opt/trn_rl_repo")

import numpy as np
import ml_dtypes

B, T, C = 4, 2048, 1024
H, D = 16, 64
NCORES = 8
HPC = 8  # heads per core

TRACE = False
LAST_EXEC_NS = None

_cache = {}


def _build():
    if "nc" in _cache:
        return _cache["nc"]
    import concourse.bass as bass  # noqa: F401
    import concourse.mybir as mybir
    from concourse import bacc, tile

    bf16 = mybir.dt.bfloat16
    f32 = mybir.dt.float32
    AF = mybir.ActivationFunctionType

    nc = bacc.Bacc(
        "TRN2", target_bir_lowering=False, debug=False, num_devices=NCORES
    )

    xT = nc.declare_dram_parameter("xT", [C, T], bf16, isOutput=False)
    wqk = nc.declare_dram_parameter("wqk", [C, 1024], bf16, isOutput=False)
    wv = nc.declare_dram_parameter("wv", [C, 512], bf16, isOutput=False)
    wp = nc.declare_dram_parameter("wp", [512, C], bf16, isOutput=False)
    mk = nc.declare_dram_parameter("mk", [128, 128], bf16, isOutput=False)
    out = nc.declare_dram_parameter("out", [T, C], f32, isOutput=True)

    KT = C // 128  # 8 contraction tiles for qkv
    TT = T // 128  # 16 s-blocks / t-tiles
    NR = T // 512  # 4 t-ranges of 512

    with tile.TileContext(nc) as tc:
        with (
            tc.tile_pool(name="wpool", bufs=1) as wpool,
            tc.tile_pool(name="big", bufs=1) as big,
            tc.tile_pool(name="pwork", bufs=4) as pwork,
            tc.tile_pool(name="owork", bufs=3) as owork,
            tc.tile_pool(name="rwork", bufs=4) as rwork,
            tc.tile_pool(name="psA", bufs=3, space="PSUM") as psA,
            tc.tile_pool(name="psY", bufs=1, space="PSUM") as psY,
        ):
            dma = nc.default_dma_engine

            # ---- loads (k-interleaved so the first matmuls start early) ----
            mask = wpool.tile([128, 128], bf16, tag="mask")
            dma.dma_start(mask[:], mk[:])
            # preload the exp table-set during startup DMA
            dum = rwork.tile([1, 8], bf16, tag="dum")
            nc.scalar.activation(dum[0:1, :], mask[0:1, 0:8], AF.Exp)
            xt, wqk_t, wv_t = [], [], []
            xTr = xT.rearrange("(k p) t -> k p t", p=128)
            wqkr = wqk.rearrange("(k p) m -> k p m", p=128)
            wvr = wv.rearrange("(k p) m -> k p m", p=128)
            for k in range(KT):
                # spread the startup loads across three queue engines --
                # a single HWDGE queue (~20 GB/s) would pace the first
                # third of the kernel
                t_ = wpool.tile([128, T], bf16, tag=f"xt{k}", name=f"xt{k}")
                dma.dma_start(t_[:], xTr[k])
                xt.append(t_)
                t_ = wpool.tile([128, 1024], bf16, tag=f"wqk{k}", name=f"wqk{k}")
                dma.dma_start(t_[:], wqkr[k])
                wqk_t.append(t_)
                t_ = wpool.tile([128, 512], bf16, tag=f"wv{k}", name=f"wv{k}")
                dma.dma_start(t_[:], wvr[k])
                wv_t.append(t_)
            wp_t = []
            wpr = wp.rearrange("(k p) m -> k p m", p=128)
            for k in range(4):
                t_ = wpool.tile([128, 1024], bf16, tag=f"wp{k}", name=f"wp{k}")
                dma.dma_start(t_[:], wpr[k])
                wp_t.append(t_)

            # ---- QT/KT per-m tiles (m 0-3 = Q heads 2m,2m+1; 4-7 = K) and
            # V-with-ones per-tm2 tiles are emitted as units, interleaved
            # into the attention stream below so the Scalar engine (exp)
            # starts early and QKV matmuls fill the PE slack.
            qkt_t = [
                big.tile([128, T], bf16, tag=f"qkt{m}", name=f"qkt{m}")
                for m in range(8)
            ]
            vones_t = [
                big.tile(
                    [128, 2, HPC, 65], bf16, tag=f"vones{tm2}", name=f"vones{tm2}"
                )
                for tm2 in range(TT // 2)
            ]

            def emit_qkt_half(m, n2):
                # one psum tile; 2 consecutive matmuls share each stationary
                # weight (k-outer) so LDWEIGHTS bubbles amortize
                qm = qkt_t[m]
                ps = psA.tile([128, 1024], f32, tag="ps", name=f"qk{m}{n2}")
                for k in range(KT):
                    for half in range(2):
                        n = 2 * n2 + half
                        nc.tensor.matmul(
                            ps[:, 512 * half : 512 * half + 512],
                            wqk_t[k][:, m * 128 : (m + 1) * 128],
                            xt[k][:, n * 512 : (n + 1) * 512],
                            start=(k == 0),
                            stop=(k == KT - 1),
                        )
                nc.vector.tensor_copy(
                    qm[:, n2 * 1024 : (n2 + 1) * 1024], ps[:]
                )

            def emit_v(tm2):
                vt = vones_t[tm2]
                nc.gpsimd.memset(vt[:, :, :, 64], 1.0)
                ps = psA.tile([128, 1024], f32, tag="ps", name=f"v{tm2}")
                for k in range(KT):
                    for half in range(2):
                        tm = 2 * tm2 + half
                        nc.tensor.matmul(
                            ps[:, 512 * half : 512 * half + 512],
                            xt[k][:, tm * 128 : (tm + 1) * 128],
                            wv_t[k][:],
                            start=(k == 0),
                            stop=(k == KT - 1),
                        )
                for half in range(2):
                    nc.vector.tensor_copy(
                        vt[:, half, :, 0:64],
                        ps[:, 512 * half : 512 * half + 512].rearrange(
                            "p (h d) -> p h d", h=HPC
                        ),
                    )

            # ---- attention + interleaved projection ----
            # yt_n[n][64*(h%2)+d, h//2, tl] = y_h[512n+tl, d] / l_h[512n+tl]
            yt_n = [
                big.tile([128, 4, 512], bf16, tag=f"ytn{n}", name=f"ytn{n}")
                for n in range(NR)
            ]

            proj_queue = []

            def mk_proj(n, tl, n2):
                # half-width projection unit: shorter PSUM-slot hold so the
                # interleaved pops perturb the score pipeline less
                def f():
                    tt = 4 * n + tl
                    pp = psA.tile([128, 1024], f32, tag="ps", name=f"prj{tt}{n2}")
                    for k4 in range(4):
                        nc.tensor.matmul(
                            pp[:, 0:512],
                            yt_n[n][:, k4, tl * 128 : (tl + 1) * 128],
                            wp_t[k4][:, n2 * 512 : (n2 + 1) * 512],
                            start=(k4 == 0),
                            stop=(k4 == 3),
                        )
                    o_s = owork.tile([128, 512], f32, tag="osb", name=f"os{tt}{n2}")
                    nc.vector.tensor_copy(o_s[:], pp[:, 0:512])
                    dma.dma_start(
                        out[
                            tt * 128 : (tt + 1) * 128,
                            n2 * 512 : (n2 + 1) * 512,
                        ],
                        o_s[:],
                    )

                return f

            blk = {"count": 0}

            def maybe_proj():
                if proj_queue and blk["count"] % 4 == 0:
                    proj_queue.pop(0)()

            defer = []  # deferred AV-matmul / normalization emissions that
            # carry the score->AV skew across pipeline boundaries so the PE
            # never drains waiting for the last exp of a pipeline

            def att(n, j):
                if True:
                    mq = j
                    mk_ = 4 + j
                    ys = [
                        psY.tile([65, 512], f32, tag=f"yt{hh}", name=f"ys{hh}")
                        for hh in range(2)
                    ]
                    nsb = 4 * n + 4
                    for sb in range(nsb):
                        smin = 128 * sb
                        t0 = max(0, smin - 512 * n)
                        ps = psA.tile([128, 1024], f32, tag="ps")
                        for hh in range(2):
                            pq = 64 * hh
                            nc.tensor.matmul(
                                ps[:, 512 * hh + t0 : 512 * hh + 512],
                                qkt_t[mk_][pq : pq + 64, smin : smin + 128],
                                qkt_t[mq][
                                    pq : pq + 64,
                                    512 * n + t0 : 512 * (n + 1),
                                ],
                                start=True,
                                stop=True,
                                skip_group_check=True,
                            )
                        p = pwork.tile([128, 1024], bf16, tag="p")
                        nc.scalar.activation(
                            p[:].rearrange("q (hh t) -> q hh t", hh=2)[
                                :, :, t0:512
                            ],
                            ps[:].rearrange("q (hh t) -> q hh t", hh=2)[
                                :, :, t0:512
                            ],
                            AF.Exp,
                            scale=0.125,
                        )
                        if sb // 4 == n:
                            # diagonal block: zero s>t half (exact, post-exp)
                            nc.vector.tensor_mul(
                                p[:].rearrange("q (hh t) -> q hh t", hh=2)[
                                    :, :, t0 : t0 + 128
                                ],
                                p[:].rearrange("q (hh t) -> q hh t", hh=2)[
                                    :, :, t0 : t0 + 128
                                ],
                                mask[:, None, :].broadcast_to([128, 2, 128]),
                            )
                        while defer:
                            defer.pop(0)()

                        def mk_yt(sb_, t0_, p_, ys_, j_, n_):
                            def f():
                                for hh in range(2):
                                    nc.tensor.matmul(
                                        ys_[hh][:, t0_:512],
                                        vones_t[sb_ // 2][
                                            :, sb_ % 2, 2 * j_ + hh, :
                                        ],
                                        p_[:, 512 * hh + t0_ : 512 * hh + 512],
                                        start=(sb_ == 0),
                                        stop=(sb_ == 4 * n_ + 3),
                                        skip_group_check=True,
                                    )
                                blk["count"] += 1
                                maybe_proj()

                            return f

                        defer.append(mk_yt(sb, t0, p, ys, j, n))
                    defer.append(mk_norm(n, j, ys))

            def mk_norm(n, j, ys):
                def f():
                    for hh in range(2):
                        # free the PSUM bank fast with a copy, then broadcast
                        # l, reciprocal on 64 lanes, normalize -- all off the
                        # PE/ACT critical path
                        ysb = rwork.tile([65, 512], f32, tag="ysb")
                        nc.vector.tensor_copy(ysb[:], ys[hh][:])
                        ls = rwork.tile([1, 512], f32, tag="ls")
                        nc.vector.tensor_copy(ls[0:1, :], ysb[64:65, :])
                        lr = rwork.tile([1, 512], f32, tag="lr")
                        nc.vector.reciprocal_approx_fast(
                            out=lr[0:1, :], in_=ls[0:1, :]
                        )
                        rb = rwork.tile([64, 512], f32, tag="rb")
                        nc.gpsimd.partition_broadcast(rb[:], lr[0:1, :])
                        nc.vector.tensor_mul(
                            yt_n[n][64 * hh : 64 * hh + 64, j, :],
                            ysb[0:64, :],
                            rb[:],
                        )

                return f

            # Emission order: feed each attention pipeline just in time.
            # att(0/1,*) touch only the first 1024 columns of QT/KT, so the
            # n2=1 QKT halves and late V units are deferred into the late,
            # ACT-paced region where the PE has slack.
            for j in range(4):
                emit_qkt_half(j, 0)
                emit_qkt_half(4 + j, 0)
                if j < 2:
                    emit_v(2 * j)
                    emit_v(2 * j + 1)
                att(0, j)
                if j >= 1:
                    att(1, j - 1)
            att(1, 3)
            while defer:
                defer.pop(0)()
            for tl in range(4):
                for n2 in range(2):
                    proj_queue.append(mk_proj(0, tl, n2))
                    proj_queue.append(mk_proj(1, tl, n2))
            for j in range(4):
                if j == 0:
                    emit_v(4)
                    emit_v(5)
                emit_qkt_half(j, 1)
                emit_qkt_half(4 + j, 1)
                att(2, j)
            while defer:
                defer.pop(0)()
            for tl in range(4):
                for n2 in range(2):
                    proj_queue.append(mk_proj(2, tl, n2))
            for j in range(4):
                if j == 0:
                    emit_v(6)
                    emit_v(7)
                att(3, j)
            while defer:
                defer.pop(0)()
            for tl in range(4):
                for n2 in range(2):
                    proj_queue.append(mk_proj(3, tl, n2))
            while proj_queue:
                proj_queue.pop(0)()

    nc.compile()
    _cache["nc"] = nc
    return nc


def kernel(x, w_qkv, w_proj, b_proj):
    global LAST_EXEC_NS
    from concourse.bass_utils import run_bass_kernel_spmd

    x = np.asarray(x)
    w_qkv = np.asarray(w_qkv)
    w_proj = np.asarray(w_proj)
    b_proj = np.asarray(b_proj)

    nc = _build()
    bf = ml_dtypes.bfloat16
    # mask[s, t] = 1 where t >= s (keep), 0 where s > t (causal-masked)
    maskt = np.triu(np.ones((128, 128), np.float32)).astype(bf)

    in_maps = []
    for core in range(NCORES):
        b, hg = core // 2, core % 2
        cs = 512 * hg
        in_maps.append(
            {
                "xT": np.ascontiguousarray(x[b].T.astype(bf)),
                "wqk": np.ascontiguousarray(
                    np.concatenate(
                        [w_qkv[:, cs : cs + 512], w_qkv[:, 1024 + cs : 1536 + cs]],
                        axis=1,
                    ).astype(bf)
                ),
                "wv": np.ascontiguousarray(
                    w_qkv[:, 2048 + cs : 2560 + cs].astype(bf)
                ),
                "wp": np.ascontiguousarray(w_proj[cs : cs + 512, :].astype(bf)),
                "mk": maskt,
            }
        )

    res = run_bass_kernel_spmd(nc, in_maps, list(range(NCORES)), trace=TRACE)
    LAST_EXEC_NS = res.exec_time_ns
    results = res.results

    outv = np.empty((B, T, C), np.float32)
    for b in range(B):
        outv[b] = (
            results[2 * b]["out"].astype(np.float32)
            + results[2 * b + 1]["out"].astype(np.float32)
            + b_proj[None, :].astype(np.float32)
        )
    return outv


# revision 43
# speedup vs baseline: 1.0319x; 1.0313x over previous
"""Causal multi-head attention (B=4,T=2048,C=1024,H=16,D=64) on 8 TRN2 cores.

Sharding: core = 2*b + hg  (b = batch 0..3, hg = head-group 0..1, 8 heads each).
Each core computes, for its batch b and its 8 heads:
  QT,KT  = (x_b @ Wq|Wk)^T      via matmul(lhsT=w_cols, rhs=x_b^T)
  V      = x_b @ Wv             (natural layout, lhsT=x_b^T tiles)
  pT     = exp(KT_h^T Q_h / 8)  (transposed scores, causal blocks only,
                                 no max-subtraction: scores are ~N(0,1/3))
  [yT;l] = [V|1]^T @ pT         (fused attention output + softmax denom)
  yT_n   = yT * (1/l)           (broadcast + DVE multiply)
  part   = Y^T.T @ Wproj_rows   (yT is directly the lhsT for the projection)
Host: out[b] = part[2b] + part[2b+1] + b_proj  (tensor-parallel unshard).

Head pairs (2j, 2j+1) are processed together: their K=64 score matmuls sit
at partition bases 0/64 (disjoint PE row groups, disjoint PSUM banks) so
they run concurrently, and one wide ACT does exp for both. The AV matmuls
are deferred one block behind score+exp (carried across pipeline
boundaries) so neither engine drains. QKV/V matmul units and projection
tile-groups are interleaved into the attention stream just-in-time, which
keeps the TensorEngine -- the binding resource at ~216ns per 128x512 bf16
matmul -- issue-saturated from the first DMA arrival to the tail.
"""

import sys

sys.path.insert(0, "/opt/trn_rl_repo")

import numpy as np
import ml_dtypes

B, T, C = 4, 2048, 1024
H, D = 16, 64
NCORES = 8
HPC = 8  # heads per core

TRACE = False
LAST_EXEC_NS = None

_cache = {}


def _build():
    if "nc" in _cache:
        return _cache["nc"]
    import concourse.bass as bass  # noqa: F401
    import concourse.mybir as mybir
    from concourse import bacc, tile

    bf16 = mybir.dt.bfloat16
    f32 = mybir.dt.float32
    AF = mybir.ActivationFunctionType

    nc = bacc.Bacc(
        "TRN2", target_bir_lowering=False, debug=False, num_devices=NCORES
    )

    xT = nc.declare_dram_parameter("xT", [C, T], bf16, isOutput=False)
    wqk = nc.declare_dram_parameter("wqk", [C, 1024], bf16, isOutput=False)
    wv = nc.declare_dram_parameter("wv", [C, 512], bf16, isOutput=False)
    wp = nc.declare_dram_parameter("wp", [512, C], bf16, isOutput=False)
    mk = nc.declare_dram_parameter("mk", [128, 128], bf16, isOutput=False)
    out = nc.declare_dram_parameter("out", [T, C], f32, isOutput=True)

    KT = C // 128  # 8 contraction tiles for qkv
    TT = T // 128  # 16 s-blocks / t-tiles
    NR = T // 512  # 4 t-ranges of 512

    with tile.TileContext(nc) as tc:
        with (
            tc.tile_pool(name="wpool", bufs=1) as wpool,
            tc.tile_pool(name="big", bufs=1) as big,
            tc.tile_pool(name="pwork", bufs=4) as pwork,
            tc.tile_pool(name="owork", bufs=3) as owork,
            tc.tile_pool(name="rwork", bufs=4) as rwork,
            tc.tile_pool(name="psA", bufs=3, space="PSUM") as psA,
            tc.tile_pool(name="psY", bufs=1, space="PSUM") as psY,
        ):
            dma = nc.default_dma_engine

            # ---- loads (k-interleaved so the first matmuls start early) ----
            mask = wpool.tile([128, 128], bf16, tag="mask")
            dma.dma_start(mask[:], mk[:])
            # preload the exp table-set during startup DMA
            dum = rwork.tile([1, 8], bf16, tag="dum")
            nc.scalar.activation(dum[0:1, :], mask[0:1, 0:8], AF.Exp)
            xt, wqk_t, wv_t = [], [], []
            xTr = xT.rearrange("(k p) t -> k p t", p=128)
            wqkr = wqk.rearrange("(k p) m -> k p m", p=128)
            wvr = wv.rearrange("(k p) m -> k p m", p=128)
            for k in range(KT):
                # spread the startup loads across three queue engines --
                # a single HWDGE queue (~20 GB/s) would pace the first
                # third of the kernel
                t_ = wpool.tile([128, T], bf16, tag=f"xt{k}", name=f"xt{k}")
                dma.dma_start(t_[:], xTr[k])
                xt.append(t_)
                t_ = wpool.tile([128, 1024], bf16, tag=f"wqk{k}", name=f"wqk{k}")
                dma.dma_start(t_[:], wqkr[k])
                wqk_t.append(t_)
                t_ = wpool.tile([128, 512], bf16, tag=f"wv{k}", name=f"wv{k}")
                dma.dma_start(t_[:], wvr[k])
                wv_t.append(t_)
            wp_t = []
            wpr = wp.rearrange("(k p) m -> k p m", p=128)
            for k in range(4):
                t_ = wpool.tile([128, 1024], bf16, tag=f"wp{k}", name=f"wp{k}")
                dma.dma_start(t_[:], wpr[k])
                wp_t.append(t_)

            # ---- QT/KT per-m tiles (m 0-3 = Q heads 2m,2m+1; 4-7 = K) and
            # V-with-ones per-tm2 tiles are emitted as units, interleaved
            # into the attention stream below so the Scalar engine (exp)
            # starts early and QKV matmuls fill the PE slack.
            qkt_t = [
                big.tile([128, T], bf16, tag=f"qkt{m}", name=f"qkt{m}")
                for m in range(8)
            ]
            vones_t = [
                big.tile(
                    [128, 2, HPC, 65], bf16, tag=f"vones{tm2}", name=f"vones{tm2}"
                )
                for tm2 in range(TT // 2)
            ]

            def emit_qkt_half(m, n2):
                # one psum tile; 2 consecutive matmuls share each stationary
                # weight (k-outer) so LDWEIGHTS bubbles amortize
                qm = qkt_t[m]
                ps = psA.tile([128, 1024], f32, tag="ps", name=f"qk{m}{n2}")
                for k in range(KT):
                    for half in range(2):
                        n = 2 * n2 + half
                        nc.tensor.matmul(
                            ps[:, 512 * half : 512 * half + 512],
                            wqk_t[k][:, m * 128 : (m + 1) * 128],
                            xt[k][:, n * 512 : (n + 1) * 512],
                            start=(k == 0),
                            stop=(k == KT - 1),
                        )
                nc.vector.tensor_copy(
                    qm[:, n2 * 1024 : (n2 + 1) * 1024], ps[:]
                )

            def emit_v(tm2):
                vt = vones_t[tm2]
                nc.gpsimd.memset(vt[:, :, :, 64], 1.0)
                ps = psA.tile([128, 1024], f32, tag="ps", name=f"v{tm2}")
                for k in range(KT):
                    for half in range(2):
                        tm = 2 * tm2 + half
                        nc.tensor.matmul(
                            ps[:, 512 * half : 512 * half + 512],
                            xt[k][:, tm * 128 : (tm + 1) * 128],
                            wv_t[k][:],
                            start=(k == 0),
                            stop=(k == KT - 1),
                        )
                for half in range(2):
                    nc.vector.tensor_copy(
                        vt[:, half, :, 0:64],
                        ps[:, 512 * half : 512 * half + 512].rearrange(
                            "p (h d) -> p h d", h=HPC
                        ),
                    )

            # ---- attention + interleaved projection ----
            # yt_n[n][64*(h%2)+d, h//2, tl] = y_h[512n+tl, d] / l_h[512n+tl]
            yt_n = [
                big.tile([128, 4, 512], bf16, tag=f"ytn{n}", name=f"ytn{n}")
                for n in range(NR)
            ]

            proj_queue = []

            def mk_proj(n, tl, n2):
                # half-width projection unit: shorter PSUM-slot hold so the
                # interleaved pops perturb the score pipeline less
                def f():
                    tt = 4 * n + tl
                    pp = psA.tile([128, 1024], f32, tag="ps", name=f"prj{tt}{n2}")
                    for k4 in range(4):
                        nc.tensor.matmul(
                            pp[:, 0:512],
                            yt_n[n][:, k4, tl * 128 : (tl + 1) * 128],
                            wp_t[k4][:, n2 * 512 : (n2 + 1) * 512],
                            start=(k4 == 0),
                            stop=(k4 == 3),
                        )
                    o_s = owork.tile([128, 512], f32, tag="osb", name=f"os{tt}{n2}")
                    nc.vector.tensor_copy(o_s[:], pp[:, 0:512])
                    dma.dma_start(
                        out[
                            tt * 128 : (tt + 1) * 128,
                            n2 * 512 : (n2 + 1) * 512,
                        ],
                        o_s[:],
                    )

                return f

            blk = {"count": 0}

            def maybe_proj():
                if proj_queue and blk["count"] % 4 == 0:
                    proj_queue.pop(0)()

            defer = []  # deferred AV-matmul / normalization emissions that
            # carry the score->AV skew across pipeline boundaries so the PE
            # never drains waiting for the last exp of a pipeline

            def att(n, j):
                if True:
                    mq = j
                    mk_ = 4 + j
                    ys = [
                        psY.tile([65, 512], f32, tag=f"yt{hh}", name=f"ys{hh}")
                        for hh in range(2)
                    ]
                    nsb = 4 * n + 4
                    for sb in range(nsb):
                        smin = 128 * sb
                        t0 = max(0, smin - 512 * n)
                        ps = psA.tile([128, 1024], f32, tag="ps")
                        for hh in range(2):
                            pq = 64 * hh
                            nc.tensor.matmul(
                                ps[:, 512 * hh + t0 : 512 * hh + 512],
                                qkt_t[mk_][pq : pq + 64, smin : smin + 128],
                                qkt_t[mq][
                                    pq : pq + 64,
                                    512 * n + t0 : 512 * (n + 1),
                                ],
                                start=True,
                                stop=True,
                                skip_group_check=True,
                            )
                        p = pwork.tile([128, 1024], bf16, tag="p")
                        nc.scalar.activation(
                            p[:].rearrange("q (hh t) -> q hh t", hh=2)[
                                :, :, t0:512
                            ],
                            ps[:].rearrange("q (hh t) -> q hh t", hh=2)[
                                :, :, t0:512
                            ],
                            AF.Exp,
                            scale=0.125,
                        )
                        if sb // 4 == n:
                            # diagonal block: zero s>t half (exact, post-exp)
                            nc.vector.tensor_mul(
                                p[:].rearrange("q (hh t) -> q hh t", hh=2)[
                                    :, :, t0 : t0 + 128
                                ],
                                p[:].rearrange("q (hh t) -> q hh t", hh=2)[
                                    :, :, t0 : t0 + 128
                                ],
                                mask[:, None, :].broadcast_to([128, 2, 128]),
                            )
                        while len(defer) > 1:
                            defer.pop(0)()

                        def mk_yt(sb_, t0_, p_, ys_, j_, n_):
                            def f():
                                for hh in range(2):
                                    nc.tensor.matmul(
                                        ys_[hh][:, t0_:512],
                                        vones_t[sb_ // 2][
                                            :, sb_ % 2, 2 * j_ + hh, :
                                        ],
                                        p_[:, 512 * hh + t0_ : 512 * hh + 512],
                                        start=(sb_ == 0),
                                        stop=(sb_ == 4 * n_ + 3),
                                        skip_group_check=True,
                                    )
                                blk["count"] += 1
                                maybe_proj()

                            return f

                        defer.append(mk_yt(sb, t0, p, ys, j, n))
                    defer.append(mk_norm(n, j, ys))

            def mk_norm(n, j, ys):
                def f():
                    for hh in range(2):
                        # free the PSUM bank fast with a copy, then broadcast
                        # l, reciprocal on 64 lanes, normalize -- all off the
                        # PE/ACT critical path
                        ysb = rwork.tile([65, 512], f32, tag="ysb")
                        nc.vector.tensor_copy(ysb[:], ys[hh][:])
                        ls = rwork.tile([1, 512], f32, tag="ls")
                        nc.vector.tensor_copy(ls[0:1, :], ysb[64:65, :])
                        lr = rwork.tile([1, 512], f32, tag="lr")
                        nc.vector.reciprocal_approx_fast(
                            out=lr[0:1, :], in_=ls[0:1, :]
                        )
                        rb = rwork.tile([64, 512], f32, tag="rb")
                        nc.gpsimd.partition_broadcast(rb[:], lr[0:1, :])
                        nc.vector.tensor_mul(
                            yt_n[n][64 * hh : 64 * hh + 64, j, :],
                            ysb[0:64, :],
                            rb[:],
                        )

                return f

            # Emission order: feed each attention pipeline just in time.
            # att(0/1,*) touch only the first 1024 columns of QT/KT, so the
            # n2=1 QKT halves and late V units are deferred into the late,
            # ACT-paced region where the PE has slack.
            for j in range(4):
                emit_qkt_half(j, 0)
                emit_qkt_half(4 + j, 0)
                if j < 2:
                    emit_v(2 * j)
                    emit_v(2 * j + 1)
                att(0, j)
                if j >= 1:
                    att(1, j - 1)
            att(1, 3)
            while defer:
                defer.pop(0)()
            for tl in range(4):
                for n2 in range(2):
                    proj_queue.append(mk_proj(0, tl, n2))
                    proj_queue.append(mk_proj(1, tl, n2))
            for j in range(4):
                if j == 0:
                    emit_v(4)
                    emit_v(5)
                emit_qkt_half(j, 1)
                emit_qkt_half(4 + j, 1)
                att(2, j)
            while defer:
                defer.pop(0)()
            for tl in range(4):
                for n2 in range(2):
                    proj_queue.append(mk_proj(2, tl, n2))
            for j in range(4):
                if j == 0:
                    emit_v(6)
                    emit_v(7)
                att(3, j)
            while defer:
                defer.pop(0)()
            for tl in range(4):
                for n2 in range(2):
                    proj_queue.append(mk_proj(3, tl, n2))
            while proj_queue:
                proj_queue.pop(0)()

    nc.compile()
    _cache["nc"] = nc
    return nc


def kernel(x, w_qkv, w_proj, b_proj):
    global LAST_EXEC_NS
    from concourse.bass_utils import run_bass_kernel_spmd

    x = np.asarray(x)
    w_qkv = np.asarray(w_qkv)
    w_proj = np.asarray(w_proj)
    b_proj = np.asarray(b_proj)

    nc = _build()
    bf = ml_dtypes.bfloat16
    # mask[s, t] = 1 where t >= s (keep), 0 where s > t (causal-masked)
    maskt = np.triu(np.ones((128, 128), np.float32)).astype(bf)

    in_maps = []
    for core in range(NCORES):
        b, hg = core // 2, core % 2
        cs = 512 * hg
        in_maps.append(
            {
                "xT": np.ascontiguousarray(x[b].T.astype(bf)),
                "wqk": np.ascontiguousarray(
                    np.concatenate(
                        [w_qkv[:, cs : cs + 512], w_qkv[:, 1024 + cs : 1536 + cs]],
                        axis=1,
                    ).astype(bf)
                ),
                "wv": np.ascontiguousarray(
                    w_qkv[:, 2048 + cs : 2560 + cs].astype(bf)
                ),
                "wp": np.ascontiguousarray(w_proj[cs : cs + 512, :].astype(bf)),
                "mk": maskt,
            }
        )

    res = run_bass_kernel_spmd(nc, in_maps, list(range(NCORES)), trace=TRACE)
    LAST_EXEC_NS = res.exec_time_ns
    results = res.results

    outv = np.empty((B, T, C), np.float32)
    for b in range(B):
        outv[b] = (
            results[2 * b]["out"].astype(np.float32)
            + results[2 * b + 1]["out"].astype(np.float32)
            + b_proj[None, :].astype(np.float32)
        )
    return outv
